# revision 11
# baseline (speedup 1.0000x reference)
"""Trainium2 Bass kernel: 3x3 sliding-window variance (zero-padded, stride 1).

Input  x: (8, 32, 512, 512) float32
Output  : (8, 32, 512, 512) float32,  var = E[x^2] - E[x]^2 over each 3x3
          window (divisor 9 everywhere, zero padding).

Sharding: batch dim across the 8 cores (core i gets x[i], no communication).

v5 design (vs v3, which used full-width [128,128] band matmuls):
  - Host casts x to fp16 and pre-blocks it into overlapping 32-row windows
    x2[C, 17, 32, W] (window t = input rows 30t..30t+31): input HBM traffic
    is 17 MB/core (vs 32 MB fp32) and every DMA is one 3-dim AP whose SBUF
    side keeps a flat partition range (partition-split SBUF APs break the
    tile framework's DMA dependency tracking - found the hard way).
  - PE work runs as 32x32 sub-array tile-matmuls (tile_position row/col
    tiling): 16 tiles (4 windows x 4 images) concurrently, each a [32,31]
    band matrix producing 30/31 output rows, x3 column-shifted accumulating
    matmuls for the horizontal 3-tap sum.  This lifts PE output bandwidth
    ~4x vs one 128-wide band matmul that leaves most of the array
    multiplying zeros.
  - PSUM: partition slice 32m = image m, bank b = window b, so mean^2 and
    the final subtract each cover [128, 2048] in ONE instruction.
  - Per (quad, set): 1 in-DMA, square on an ACT/DVE column split, 2x48
    tile-matmuls (x and x^2) into ps_x/ps_sq, ACT mean^2 = Square(s*ps_x),
    DVE scalar_tensor_tensor var = s*ps_sq - mean^2, 1 store.
  - Output layout y2[C, 32, 17, W] (window-blocked, garbage rows included)
    so each store is one flat-SBUF AP; the host unpacks to [C, H, W].
  - The 17th (bottom) window of all 32 images runs as 2 "mega-tail" passes
    of 16 images each.
"""

import os

import numpy as np

import concourse.bacc as bacc
import concourse.bass as bass
import concourse.mybir as mybir
import concourse.tile as tile
from concourse.bass_utils import run_bass_kernel_spmd

F32 = mybir.dt.float32
F16 = mybir.dt.float16

B, C, H, W = 8, 32, 512, 512
NT = 17                   # row windows per image
CW = 7.0 / 64.0           # band weight, exact in fp16
SCALE = 1.0 / (9.0 * CW)  # rescale applied in fp32 at evacuation
NA = 1024                 # square-op columns done on ACT (rest on DVE)


def _weight_arrays():
    # Window t covers input rows 30t..30t+31 (t = 0..16).
    # w_top (t=0): partition k = image row k; out m = image row m,
    #   taps k in {m-1, m, m+1} clipped at 0 (row -1 is zero padding). M=31.
    # w_int (t=1..15): out m = image row 30t+1+m, taps k = m, m+1, m+2. M=30.
    # w_bot (t=16): out m = image row 481+m, taps k = m, m+1, m+2 clipped at
    #   31 (row 512 is zero padding). M=31.
    w_top = np.zeros((32, 31), np.float32)
    for m in range(31):
        for k in (m - 1, m, m + 1):
            if 0 <= k < 32:
                w_top[k, m] = CW
    w_int = np.zeros((32, 31), np.float32)
    for m in range(30):
        for k in (m, m + 1, m + 2):
            w_int[k, m] = CW
    w_bot = np.zeros((32, 31), np.float32)
    for m in range(31):
        for k in (m, m + 1, m + 2):
            if k < 32:
                w_bot[k, m] = CW
    # SBUF weight tiles [128, 31]: the same matrix replicated in all 4
    # partition groups (except set0's group 0, which holds w_top).
    ws0 = np.concatenate([w_top] + [w_int] * 3, axis=0)
    wint = np.concatenate([w_int] * 4, axis=0)
    wbot = np.concatenate([w_bot] * 4, axis=0)
    return (ws0.astype(np.float16), wint.astype(np.float16),
            wbot.astype(np.float16))


def _square(nc, sq, xt):
    # sq = xt * xt, split across ACT (cols 0..NA) and DVE (cols NA..2048).
    nc.scalar.activation(sq[:, 0:NA], xt[:, 0:NA],
                         mybir.ActivationFunctionType.Square)
    nc.vector.tensor_tensor(sq[:, NA:], xt[:, NA:], xt[:, NA:],
                            mybir.AluOpType.mult)


def _tile_mms(nc, ps, wsel, src, tap):
    """One tap wave: 16 tile-matmuls (4 row-groups x 4 col-groups).

    wsel(b) -> (weights tile, M); row-group 32b holds window b's moving
    rows, col-group 32m image m's output partitions.  tap 0 = center
    (start=True), 1 = left, 2 = right (stop=True).
    """
    for b in range(4):
        wt, M = wsel(b)
        lhsT = wt[32 * b:32 * b + 32, 0:M]
        for m in range(4):
            if tap == 0:
                nc.tensor.matmul(ps[32 * m:32 * m + M, 512 * b:512 * b + 512],
                                 lhsT, src[32 * b:32 * b + 32,
                                           512 * m:512 * m + 512],
                                 start=True, stop=False,
                                 tile_position=(32 * b, 32 * m))
            elif tap == 1:
                nc.tensor.matmul(ps[32 * m:32 * m + M, 512 * b + 1:512 * b + 512],
                                 lhsT, src[32 * b:32 * b + 32,
                                           512 * m:512 * m + 511],
                                 start=False, stop=False,
                                 tile_position=(32 * b, 32 * m))
            else:
                nc.tensor.matmul(ps[32 * m:32 * m + M, 512 * b:512 * b + 511],
                                 lhsT, src[32 * b:32 * b + 32,
                                           512 * m + 1:512 * m + 512],
                                 start=False, stop=True,
                                 tile_position=(32 * b, 32 * m))


def _emit_pair(nc, xt, sq, wsel, ps_x, ps_sq, m2, outt):
    _square(nc, sq, xt)
    for tap in range(3):
        _tile_mms(nc, ps_x, wsel, xt, tap)
    for tap in range(3):
        _tile_mms(nc, ps_sq, wsel, sq, tap)
    nc.scalar.activation(m2[:, :], ps_x[0:128, 0:2048],
                         mybir.ActivationFunctionType.Square, scale=SCALE)
    nc.vector.scalar_tensor_tensor(
        outt[:, :], ps_sq[0:128, 0:2048], SCALE, m2[:, :],
        mybir.AluOpType.mult, mybir.AluOpType.subtract)


def _emit_main(nc, x2, y2, ws0, wint, j, s, xpool, sqpool, m2pool, outpool,
               pspool):
    """Quad j (images 4j..4j+3), set s (windows 4s..4s+3)."""
    xt = xpool.tile([128, 2048], F16, tag="xt")
    # X[32b+q, 512m+w] = x2[4j+m, 4s+b, q, w]; ONE dma, flat SBUF partitions
    nc.sync.dma_start(
        xt[:].rearrange("p (m w) -> p m w", m=4),
        x2[4 * j:4 * j + 4, 4 * s:4 * s + 4, :, :]
        .rearrange("m b q w -> (b q) m w"),
    )
    sq = sqpool.tile([128, 2048], F16, tag="sq")
    ps_x = pspool.tile([128, 2048], F32, tag="ps")
    ps_sq = pspool.tile([128, 2048], F32, tag="ps")
    m2 = m2pool.tile([128, 2048], F16, tag="m2")
    outt = outpool.tile([128, 2048], F16, tag="outt")

    def wsel(b):
        if s == 0 and b == 0:
            return ws0, 31
        return wint, 30

    _emit_pair(nc, xt, sq, wsel, ps_x, ps_sq, m2, outt)

    # One store: y2[img, q, window, w] <- outt[32m+q, 512b+w]
    nc.scalar.dma_start(
        y2[4 * j:4 * j + 4, :, 4 * s:4 * s + 4, :]
        .rearrange("m q b w -> (m q) (b w)"),
        outt[:, :],
    )


def _emit_tail(nc, x2, y2, wbot, h, xpool, sqpool, m2pool, outpool, pspool):
    """Mega-tail: bottom window (rows 480..511 -> outs 481..511) of 16
    images.  XT[32g+q, 512m+w] = x2[16h+4g+m, 16, q, w]."""
    i0 = 16 * h
    xt = xpool.tile([128, 2048], F16, tag="xt")
    for g in range(4):
        nc.sync.dma_start(
            xt[32 * g:32 * g + 32, :].rearrange("q (m w) -> q m w", m=4),
            x2[i0 + 4 * g:i0 + 4 * g + 4, NT - 1, :, :]
            .rearrange("m q w -> q m w"),
        )
    sq = sqpool.tile([128, 2048], F16, tag="sq")
    ps_x = pspool.tile([128, 2048], F32, tag="ps")
    ps_sq = pspool.tile([128, 2048], F32, tag="ps")
    m2 = m2pool.tile([128, 2048], F16, tag="m2")
    outt = outpool.tile([128, 2048], F16, tag="outt")

    def wsel(g):
        return wbot, 31

    _emit_pair(nc, xt, sq, wsel, ps_x, ps_sq, m2, outt)

    for g in range(4):
        nc.scalar.dma_start(
            y2[i0 + 4 * g:i0 + 4 * g + 4, :, NT - 1, :]
            .rearrange("m q w -> (m q) w"),
            outt[:, 512 * g:512 * g + 512],
        )


def build_program():
    nc = bacc.Bacc("TRN2", target_bir_lowering=False, debug=False)
    x2 = nc.declare_dram_parameter("x2", [C, NT, 32, W], F16, isOutput=False)
    a_s0 = nc.declare_dram_parameter("a_s0", [128, 31], F16, isOutput=False)
    a_int = nc.declare_dram_parameter("a_int", [128, 31], F16, isOutput=False)
    a_bot = nc.declare_dram_parameter("a_bot", [128, 31], F16, isOutput=False)
    y2 = nc.declare_dram_parameter("y2", [C, 32, NT, W], F16, isOutput=True)

    with tile.TileContext(nc) as tc:
        with (
            tc.tile_pool(name="const", bufs=1) as cpool,
            tc.tile_pool(name="xp", bufs=3) as xpool,
            tc.tile_pool(name="sqp", bufs=3) as sqpool,
            tc.tile_pool(name="m2p", bufs=2) as m2pool,
            tc.tile_pool(name="outp", bufs=2) as outpool,
            tc.tile_pool(name="ps", bufs=2, space="PSUM") as pspool,
        ):
            ws0 = cpool.tile([128, 31], F16, tag="ws0")
            wint = cpool.tile([128, 31], F16, tag="wint")
            wbot = cpool.tile([128, 31], F16, tag="wbot")
            nc.sync.dma_start(ws0[:], a_s0[:])
            nc.sync.dma_start(wint[:], a_int[:])
            nc.sync.dma_start(wbot[:], a_bot[:])

            def emit_body():
                for j in range(C // 4):
                    for s in range(4):
                        _emit_main(nc, x2, y2, ws0, wint, j, s, xpool, sqpool,
                                   m2pool, outpool, pspool)
                for h in range(2):
                    _emit_tail(nc, x2, y2, wbot, h, xpool, sqpool, m2pool,
                               outpool, pspool)

            repeat = int(os.environ.get("CHVAR_BENCH_REPEAT", "0"))
            if repeat > 1:
                with tc.For_i(0, repeat, 1):
                    emit_body()
            else:
                emit_body()
    return nc


_CACHE = {}


def _get_program():
    if "nc" not in _CACHE:
        nc = build_program()
        nc.finalize()
        _CACHE["nc"] = nc
    return _CACHE["nc"]


def make_in_maps(x: np.ndarray):
    x16 = np.ascontiguousarray(x, np.float32).astype(np.float16)
    # window-blocked input: x2[c, t, q, w] = x16[c, 30t+q, w]
    idx = (30 * np.arange(NT))[:, None] + np.arange(32)[None, :]  # [17, 32]
    x2 = x16[:, :, idx, :]                      # [B, C, 17, 32, W]
    a_s0, a_int, a_bot = _weight_arrays()
    return [
        {"x2": np.ascontiguousarray(x2[i]), "a_s0": a_s0, "a_int": a_int,
         "a_bot": a_bot}
        for i in range(B)
    ]


def _unpack(y2: np.ndarray) -> np.ndarray:
    # y2[img, q, window, w] fp16 -> y[img, row, w] fp32
    out = np.empty((C, H, W), np.float32)
    out[:, 0:31] = y2[:, 0:31, 0]
    mid = y2[:, 0:30, 1:16]                     # [C, 30, 15, W]
    out[:, 31:481] = mid.transpose(0, 2, 1, 3).reshape(C, 450, W)
    out[:, 481:512] = y2[:, 0:31, 16]
    return out


def kernel(x: np.ndarray, _trace: bool = False, **_ignored):
    assert x.shape == (B, C, H, W), x.shape
    nc = _get_program()
    in_maps = make_in_maps(x)
    res = run_bass_kernel_spmd(nc, in_maps, list(range(B)), trace=_trace)
    out = np.stack([_unpack(res.results[i]["y2"]) for i in range(B)], axis=0)
    if _trace:
        return out, res
    return out
